# revision 30
# baseline (speedup 1.0000x reference)
"""Trainium2 Bass kernel for nn_CICDM (Choquet-integral cognitive-diagnosis model).

Computation (see reference):
  sel = sigmoid(emb[stu_id])                       # [B, 30]
  x_k = sel[:, q_idx[:, k]]  k=0,1,2               # [B, N]
  C   = Choquet integral of (x0,x1,x2) against fuzzy measure FM(fm_vars)
  out = sigmoid(relu(relu(C@w1.T+b1)@w2.T+b2)@w3.T+b3)

Mobius/hinge form:  C = Lin + a01*r01 + a02*r02 + a12*r12 + au*wmin with
  Lin  = c0*x0 + c1*x1 + c2*x2
  r01  = relu(x0-x1), r02 = relu(x0-x2), r12 = relu(x1-x2)
  wmin = min(x0-x2, relu(x0-x1))

Structure (v3):
  - Lin never materializes: W1lin := Gc @ w1.T ([30,256], host) contracts
    sel directly into layer 1.
  - the four hinge tensors contract straight into layer 1 through four
    host-scaled copies of w1 (coefficient/sign folded per exercise):
      r01   -> a01 . w1        r02s = |a02|relu(D02) -> s02 . w1
      r12s  = relu(|a12|D12)   -> s12 . w1   (|a12| folded into the gather)
      wminu -> au . w1
    so l1 = W1lin^T sel + sum_t sum_ch (coef.w1)_t^T r_ch,t + b1 and no
    per-exercise scaling op ever runs on DVE (keeps PE dense -> HAM warm).
  - ONE 512-descriptor indirect gather (bf16 emb rows, host-cast) right
    after the sidx DMA; big weight DMAs are schedule-floored past the
    gather's transfer window so they never contend with its row reads.
  - sel strips replicate via one strided DVE copy + PE transposes;
    output is written fp16 (host casts back to f32).

Distribution: data-parallel over the batch: 8 cores x 512 rows, transposed
layout (exercises on partitions, batch on the free dim).
"""

import numpy as np

B = 4096
NCORES = 8
BL = B // NCORES          # 512 local batch
KN = 30
NOUT = 1024
NT = NOUT // 128          # 8 exercise tiles
P = 128
NG = BL // P              # 4 gather groups per core
S_N = 100000
N_WARM = 20               # PE warm-up matmuls during the DMA/gather phase

# bf16 weight tensors, staged by need-time:
#   megA (early, tiny): gather planes + W1lin
#   megB1/megB2/megC (issued after the sigmoid, once the gather is done):
#   w1 channel copies t-major [t, ch, m], then w2, w3
MA_GCAT = 0               # [128, 1024] gather planes per tile
MA_W1LIN = 1024           # [128, 128]  W1lin row-stacked (m0 @ parts 0:30, m1 @ 32:62)
MA_END = 1152
MB_TILES = 2              # w1ch tiles per megB tensor
MB_END = MB_TILES * 1024  # megB1: w1ch t0-1, megB2: w1ch t2-3
MC_W1CH = 0               # [128, 4*1024] w1ch tiles 4..7
MC_W2 = 4096              # [128, 256]
MC_W3 = 4352              # [128, 1024]
MC_END = 5376

# megaf (f32) column layout
MF_A02 = 0                # [128, 8]  |a02| activation scales
MF_B1 = 8                 # [128, 2]
MF_B2 = 10                # [128, 1]
MF_B3 = 11                # [128, 8]
MF_END = 19

_PROG_CACHE = {}


def _bf16():
    import ml_dtypes
    return np.dtype(ml_dtypes.bfloat16)


def _host_prep(q_idx, fm_vars, w1, b1, w2, b2, w3, b3):
    """Derive all per-exercise constants + weight layouts on the host."""
    q = np.asarray(q_idx).astype(np.int64)          # [N, 3]
    fm = np.asarray(fm_vars, dtype=np.float64)

    chi = np.abs(fm)
    f0, f1 = chi[0], chi[1]
    f2 = np.maximum(f0, f1) + chi[2]
    f3 = chi[3]
    f4 = np.maximum(f3, f0) + chi[4]
    f5 = np.maximum(f3, f1) + chi[5]
    FM = np.minimum(np.stack([f0, f1, f2, f3, f4, f5, np.ones_like(f0)], 0), 1.0)
    F0, F1, F2, F3, F4, F5, F6 = FM
    m0, m1, m3 = F0, F1, F3
    m2 = F2 - F0 - F1
    m4 = F4 - F0 - F3
    m5 = F5 - F1 - F3
    m6 = F6 - F2 - F4 - F5 + F0 + F1 + F3
    c0 = m0 + m2 + m4
    c1 = m1 + m5
    c2 = m3 + m6
    a01 = -(m2 + m6)
    a02 = -m4
    a12 = -m5
    au = m6

    s02 = np.where(a02 >= 0, 1.0, -1.0)
    s12 = np.where(a12 >= 0, 1.0, -1.0)
    a02abs = np.abs(a02)
    a12abs = np.abs(a12)

    n = np.arange(NOUT)
    t_i, nl = n // P, n % P
    q0, q1, q2 = q[:, 0], q[:, 1], q[:, 2]

    # gather planes: strip 0 -> D01 = x0-x1, strip 1 -> D02 = x0-x2,
    # strip 2 -> D12' = |a12|*(x1-x2)
    gcat = np.zeros((P, NOUT), dtype=np.float64)
    gcat[q0, t_i * P + nl] += 1.0
    gcat[q1, t_i * P + nl] -= 1.0
    gcat[32 + q0, t_i * P + nl] += 1.0
    gcat[32 + q2, t_i * P + nl] -= 1.0
    gcat[64 + q1, t_i * P + nl] += a12abs
    gcat[64 + q2, t_i * P + nl] -= a12abs

    w1f = np.asarray(w1, np.float64)                # [256, 1024]
    G = np.zeros((KN, NOUT), dtype=np.float64)
    G[q0, n] += c0
    G[q1, n] += c1
    G[q2, n] += c2
    W1lin = G @ w1f.T                               # [30, 256]
    w1lin_s = np.zeros((P, P), dtype=np.float64)
    w1lin_s[0:KN, :] = W1lin[:, 0:P]
    w1lin_s[32:32 + KN, :] = W1lin[:, P:2 * P]

    # w1 channel copies, t-major: [128, NT, 4, 256]
    # [nl, t, ch, m*128+mm] = coef_ch[t*128+nl] * w1[m*128+mm, t*128+nl]
    coefs = np.stack([a01, s02, s12, au], 0)        # [4, NOUT] ch order: r01, r02s, r12s, wminu
    wt = w1f.T.reshape(NT, P, 1, 256) * coefs.T.reshape(NT, P, 4, 1)
    w1ch = wt.transpose(1, 0, 2, 3).reshape(P, NT * 4 * 256)

    w2t = np.asarray(w2, np.float64).T.reshape(2, P, P)          # [m, p, o]
    w2s = w2t.transpose(1, 0, 2).reshape(P, 2 * P)
    w3s = np.asarray(w3, np.float64).T                           # [128, 1024]

    bf = _bf16()
    megA = np.ascontiguousarray(
        np.concatenate([gcat, w1lin_s], axis=1)).astype(bf)
    megB1 = np.ascontiguousarray(w1ch[:, 0:2 * 1024]).astype(bf)
    megB2 = np.ascontiguousarray(w1ch[:, 2 * 1024:4 * 1024]).astype(bf)
    megC = np.ascontiguousarray(
        np.concatenate([w1ch[:, 4 * 1024:], w2s, w3s], axis=1)).astype(bf)
    assert megA.shape == (P, MA_END) and megC.shape == (P, MC_END)

    def pcol(v):  # [NOUT] -> [128, NT]
        return v.reshape(NT, P).T

    b1c = np.asarray(b1, np.float64).reshape(2, P).T
    b2c = np.asarray(b2, np.float64).reshape(1, P).T
    b3c = pcol(np.asarray(b3, np.float64))
    megaf = np.concatenate([pcol(a02abs), b1c, b2c, b3c], axis=1)
    assert megaf.shape == (P, MF_END)
    megaf = np.ascontiguousarray(megaf).astype(np.float32)

    return dict(megA=megA, megB1=megB1, megB2=megB2, megC=megC, megaf=megaf)


def _build_program():
    """Build + compile the Bacc program (one NEFF shared by all 8 cores)."""
    key = "v3"
    if key in _PROG_CACHE:
        return _PROG_CACHE[key]

    import concourse.bacc as bacc
    import concourse.bass as bass
    import concourse.mybir as mybir
    import concourse.tile as tile
    from concourse.masks import make_identity

    f32 = mybir.dt.float32
    bf16 = mybir.dt.bfloat16
    fp16 = mybir.dt.float16
    AF = mybir.ActivationFunctionType
    ALU = mybir.AluOpType

    nc = bacc.Bacc("TRN2", target_bir_lowering=False, debug=False,
                   num_swdge_queues=4)

    emb_d = nc.dram_tensor("emb", [S_N, KN], bf16, kind="ExternalInput").ap()
    sidx_d = nc.dram_tensor("sidx", [P, NG], mybir.dt.int32, kind="ExternalInput").ap()
    megA_d = nc.dram_tensor("megA", [P, MA_END], bf16, kind="ExternalInput").ap()
    megB1_d = nc.dram_tensor("megB1", [P, MB_END], bf16, kind="ExternalInput").ap()
    megB2_d = nc.dram_tensor("megB2", [P, MB_END], bf16, kind="ExternalInput").ap()
    megC_d = nc.dram_tensor("megC", [P, MC_END], bf16, kind="ExternalInput").ap()
    megaf_d = nc.dram_tensor("megaf", [P, MF_END], f32, kind="ExternalInput").ap()
    out_d = nc.dram_tensor("out", [NOUT, BL], fp16, kind="ExternalOutput").ap()

    def mm(out, lhsT, rhs, start, stop, tile_position=None):
        nc.tensor.matmul(out, lhsT, rhs, start=start, stop=stop,
                         tile_position=tile_position)

    with tile.TileContext(nc) as tc:
        with (
            tc.tile_pool(name="const", bufs=1) as cpool,
            tc.tile_pool(name="work", bufs=3) as wpool,
            tc.tile_pool(name="outb", bufs=6) as opool,
            tc.tile_pool(name="pdiff", bufs=2, space="PSUM") as pdiff,
            tc.tile_pool(name="pl1", bufs=2, space="PSUM") as pl1,
        ):
            # ---- input DMAs: sidx first (gates the gathers), then consts ----
            sidx_s = cpool.tile([P, NG], mybir.dt.int32, tag="sidx")
            nc.sync.dma_start(sidx_s[:], sidx_d[:])
            megaf_s = cpool.tile([P, MF_END], f32, tag="megaf")
            nc.sync.dma_start(megaf_s[:], megaf_d[:])
            megA_s = cpool.tile([P, MA_END], bf16, tag="megA")
            nc.sync.dma_start(megA_s[:], megA_d[:])
            megB1_s = cpool.tile([P, MB_END], bf16, tag="megB1")
            megB2_s = cpool.tile([P, MB_END], bf16, tag="megB2")
            megC_s = cpool.tile([P, MC_END], bf16, tag="megC")

            # ---- gpsimd: the single 512-descriptor gather first, then trin
            #      memsets + identity (they fit inside the sidx DMA wait) ----
            stu4 = cpool.tile([P, NG * KN], bf16, tag="stu4")
            nc.gpsimd.indirect_dma_start(
                out=stu4[:], out_offset=None, in_=emb_d[:],
                in_offset=bass.IndirectOffsetOnAxis(ap=sidx_s[:, 0:NG], axis=0))
            trin_all = cpool.tile([P, NG * P], f32, tag="trin")
            nc.gpsimd.memset(trin_all[:], 0.0)
            identf = cpool.tile([P, P], f32, tag="identf")
            make_identity(nc, identf[:])

            # ---- PE warm-up burst while DMAs land (HAM un-throttle) ----
            warm = cpool.tile([P, BL], bf16, tag="warm")
            nc.vector.memset(warm[:], 0.0)

            def emit_warm(i):
                wps = pdiff.tile([P, BL], f32, tag="d12", name=f"wps{i}")
                mm(wps, warm[:, :P], warm[:], True, True)

            for i in range(N_WARM):
                emit_warm(i)

            # ---- sigmoid + per-group strip replication + transpose ----
            selb = cpool.tile([P, NG * KN], f32, tag="selb")
            selT3 = cpool.tile([P, BL], bf16, tag="selT3")
            nc.scalar.activation(selb[:], stu4[:], AF.Sigmoid)
            # big weight DMAs are floored past the gather's transfer window so
            # they never contend with its 512 row reads (Tile reorders plain
            # emission order, so use explicit schedule floors)
            with tc.tile_wait_until(0.0070):
                nc.scalar.dma_start(megB1_s[:], megB1_d[:])
            with tc.tile_wait_until(0.0085):
                nc.scalar.dma_start(megB2_s[:], megB2_d[:])
            with tc.tile_wait_until(0.0100):
                nc.scalar.dma_start(megC_s[:], megC_d[:])
            dst3 = (trin_all[:]
                    .rearrange("p (g x) -> p g x", g=NG)[:, :, 0:96]
                    .rearrange("p g (s k) -> p g s k", s=3)[:, :, :, 0:KN])
            src3 = (selb[:]
                    .rearrange("p (g k) -> p g k", g=NG)
                    .unsqueeze(2).broadcast_to([P, NG, 3, KN]))
            nc.vector.tensor_copy(dst3, src3)
            for g in range(NG):
                tp = pdiff.tile([P, BL], f32, tag="d01" if g % 2 == 0 else "d02",
                                name=f"tp{g}")
                nc.tensor.transpose(tp[:, 0:P], trin_all[:, g * P:(g + 1) * P],
                                    identf[:])
                dst = selT3[0:94, g * P:(g + 1) * P]
                if g % 2 == 0:
                    nc.vector.tensor_copy(dst, tp[0:94, 0:P])
                else:
                    nc.scalar.copy(dst, tp[0:94, 0:P])

            l1ps = [pl1.tile([P, BL], f32, tag="l1", name=f"l1p{m}")
                    for m in range(2)]

            # ---- Choquet tiles (software-pipelined: gathers t+1 before w1ch t) ----
            def emit_gathers(t):
                gc = megA_s[:, MA_GCAT + t * P:MA_GCAT + (t + 1) * P]
                d01 = pdiff.tile([P, BL], f32, tag="d01", name=f"d01_{t}")
                d02 = pdiff.tile([P, BL], f32, tag="d02", name=f"d02_{t}")
                d12 = pdiff.tile([P, BL], f32, tag="d12", name=f"d12_{t}")
                mm(d01, gc[0:KN, :], selT3[0:KN, :], True, True,
                   tile_position=(0, 0))
                mm(d02, gc[32:32 + KN, :], selT3[32:32 + KN, :], True, True,
                   tile_position=(32, 0))
                mm(d12, gc[64:64 + KN, :], selT3[64:64 + KN, :], True, True,
                   tile_position=(64, 0))
                return d01, d02, d12

            def w1ch_ap(t, ch, m):
                off = (t % 2) * 1024 + ch * 256 + m * P
                if t < 2:
                    return megB1_s[:, off:off + P]
                if t < 4:
                    return megB2_s[:, off:off + P]
                base = MC_W1CH + (t - 4) * 1024 + ch * 256 + m * P
                return megC_s[:, base:base + P]

            # gathers for tiles 0-1 go first (shortens the pipeline-fill
            # bubble); the Lin matmuls fill the PE while tile 0's hinges run
            diffs = {0: emit_gathers(0), 1: emit_gathers(1)}
            mm(l1ps[0], megA_s[0:KN, MA_W1LIN:MA_W1LIN + P],
               selT3[0:KN, :], True, False, tile_position=(0, 0))
            mm(l1ps[1], megA_s[32:32 + KN, MA_W1LIN:MA_W1LIN + P],
               selT3[32:32 + KN, :], True, False, tile_position=(32, 0))
            for t in range(NT):
                d01, d02, d12 = diffs.pop(t)
                r01 = wpool.tile([P, BL], bf16, tag="r01")
                nc.scalar.activation(r01[:], d01[:], AF.Relu)
                r02s = wpool.tile([P, BL], bf16, tag="r02")
                nc.scalar.activation(r02s[:], d02[:], AF.Relu,
                                     scale=megaf_s[:, MF_A02 + t:MF_A02 + t + 1])
                r12s = wpool.tile([P, BL], bf16, tag="r12")
                nc.vector.tensor_scalar_max(r12s[:], d12[:], 0.0)
                wminu = wpool.tile([P, BL], bf16, tag="wm")
                nc.vector.scalar_tensor_tensor(
                    wminu[:], d02[:], 1.0, r01[:], ALU.mult, ALU.min)

                last = t == NT - 1
                for ch, rt in ((0, r01), (1, r02s), (2, r12s), (3, wminu)):
                    for m in range(2):
                        mm(l1ps[m], w1ch_ap(t, ch, m), rt[:],
                           False, last and ch == 3)
                if t + 2 < NT:
                    diffs[t + 2] = emit_gathers(t + 2)

            # ---- MLP tail ----
            h1 = cpool.tile([P, 2 * BL], bf16, tag="h1")
            nc.scalar.activation(h1[:, 0:BL], l1ps[0][:], AF.Relu,
                                 bias=megaf_s[:, MF_B1:MF_B1 + 1])
            nc.vector.tensor_scalar(h1[:, BL:2 * BL], l1ps[1][:],
                                    megaf_s[:, MF_B1 + 1:MF_B1 + 2], 0.0,
                                    ALU.add, ALU.max)

            l2p = pdiff.tile([P, BL], f32, tag="d01", name="l2p")
            mm(l2p, megC_s[:, MC_W2:MC_W2 + P], h1[:, 0:BL], True, False)
            mm(l2p, megC_s[:, MC_W2 + P:MC_W2 + 2 * P], h1[:, BL:2 * BL],
               False, True)
            h2 = cpool.tile([P, BL], bf16, tag="h2")
            nc.scalar.activation(h2[:], l2p[:], AF.Relu,
                                 bias=megaf_s[:, MF_B2:MF_B2 + 1])

            for o in range(NT):
                l3p = pdiff.tile([P, BL], f32, tag=("d02", "d12", "d01")[o % 3],
                                 name=f"l3p{o}")
                mm(l3p, megC_s[:, MC_W3 + o * P:MC_W3 + (o + 1) * P],
                   h2[:], True, True)
                osb = opool.tile([P, BL], fp16, tag="osb")
                nc.scalar.activation(osb[:], l3p[:], AF.Sigmoid,
                                     bias=megaf_s[:, MF_B3 + o:MF_B3 + o + 1])
                nc.sync.dma_start(out_d[o * P:(o + 1) * P, :], osb[:])

    nc.compile()
    _PROG_CACHE[key] = nc
    return nc


def _run(inputs, trace=False, tmpdir=None):
    from concourse import bass_utils

    nc = _build_program()

    prep = _host_prep(inputs["q_idx"], inputs["fm_vars"],
                      inputs["w1"], inputs["b1"], inputs["w2"], inputs["b2"],
                      inputs["w3"], inputs["b3"])
    emb = np.ascontiguousarray(np.asarray(inputs["emb"], np.float32).astype(_bf16()))
    stu_id = np.asarray(inputs["stu_id"]).astype(np.int32)

    in_maps = []
    for c in range(NCORES):
        sidx = np.ascontiguousarray(
            stu_id[c * BL:(c + 1) * BL].reshape(NG, P).T).astype(np.int32)
        in_maps.append(dict(emb=emb, sidx=sidx, **prep))

    if trace:
        # NTFF profiling needs the antenv.axon_hooks shim + no S3 upload.
        import sys, types
        if "antenv.axon_hooks" not in sys.modules:
            import trn_agent_boot.trn_boot as tb
            mod = types.ModuleType("antenv.axon_hooks")
            hook = tb._ntff_profile_via_ctypes("/opt/axon/libaxon_pjrt.so")
            mod.get_axon_ntff_profile_hook = lambda: hook
            mod.set_axon_ntff_profile_hook = lambda h: None
            sys.modules["antenv.axon_hooks"] = mod
        bass_utils.upload_artifacts = lambda d: d

    res = bass_utils.run_bass_kernel_spmd(
        nc, in_maps, core_ids=list(range(NCORES)), trace=trace, tmpdir=tmpdir)

    out = np.concatenate(
        [np.asarray(res.results[c]["out"]).astype(np.float32).T
         for c in range(NCORES)], axis=0)
    return np.ascontiguousarray(out), res


def kernel(**inputs):
    out, _ = _run(inputs, trace=False)
    return out
